# revision 33
# baseline (speedup 1.0000x reference)
"""Trainium2 Bass kernel for nn_Attention_63934883168998.

Math (per token t): q,k,v = x W{q,k,v}^T reshaped (16 heads, 64); scores over
HEADS: S = q k^T / 8 (16x16), A = softmax(S), out = A v -> (1024); y = out Wo^T.

Sharding: pure data parallel over the 16384 tokens -> 2048 tokens/core.
All on-chip data fp16 (PE fp16 matmul = full rate, ~5e-4 GEMM rel-err).

Per 128-token tile:
  - x^T arrives pre-transposed from the host; projections on PE with
    stationary = x^T chunk, moving = host-pretransposed W^T -> q,k,v in
    natural (token-partition) layout.
  - scores S[t,(g,h)] = sum_d q*k on the Vector engine: one broadcast-AP
    tensor_tensor (2x mode) + a log2(64)-pass pairwise-add tree (2x mode);
    softmax on ScalarE/Vector.  g-major layout so the A-scatter below has
    contiguous 32B runs.
  - AV combine on the TENSOR engine: stationary = 128x128 block-diagonal A
    for 8 tokens (K=(tau,g)), moving = v gathered to (tau,g)-partition
    layout; off-diagonal zeros kill cross-token terms.  Operands built by
    partition-scatter DMAs (plain, 32B/128B runs).
  - attention out scattered back to token-partition layout, DMA-transposed
    to feature-partition chunks, final projection on PE.
"""

import numpy as np
import ml_dtypes

BF16 = ml_dtypes.bfloat16
N_CORES = 8
HID = 1024
NH, HD = 16, 64
TILE = 128
TPC = 16384 // N_CORES      # tokens per core
NT = TPC // TILE            # token tiles per core
NCH = HID // 128            # 8 hidden chunks
NG = TILE // 8              # 16 groups of 8 tokens

_cache = {}


def _build():
    if "nc" in _cache:
        return
    import concourse.bacc as bacc
    import concourse.mybir as mybir
    from concourse import tile
    from concourse.tile import add_dep_helper

    f16 = mybir.dt.float16
    bf16 = mybir.dt.bfloat16
    f32 = mybir.dt.float32
    AX = mybir.AxisListType
    OP = mybir.AluOpType
    AF = mybir.ActivationFunctionType

    nc = bacc.Bacc("TRN2", target_bir_lowering=False, debug=False)
    xt = nc.dram_tensor("xt", (HID, TPC), f16, kind="ExternalInput").ap()
    wts = {
        n: nc.dram_tensor(n, (HID, HID), f16, kind="ExternalInput").ap()
        for n in ("wqt", "wkt", "wvt", "wot")
    }
    # block-diag 0/1 mask: mask[(b,g),(b',h)] = (b == b')
    mask_d = nc.dram_tensor("mask", (128, 128), f16, kind="ExternalInput").ap()
    ident_d = nc.dram_tensor("ident", (128, 128), f16, kind="ExternalInput").ap()
    y = nc.dram_tensor("y", (TPC, HID), f16, kind="ExternalOutput").ap()

    with tile.TileContext(nc) as tc:
        with (
            tc.tile_pool(name="wpool", bufs=1) as wpool,
            tc.tile_pool(name="work", bufs=3) as work,
            tc.tile_pool(name="prod", bufs=1) as prodp,
            tc.tile_pool(name="psum", bufs=1, space="PSUM") as pp,
        ):
            # Resident weights: chunk c = W^T[c*128:(c+1)*128, :]
            w_sb = {}
            engs = [nc.sync, nc.scalar, nc.gpsimd]
            for n in ("wqt", "wkt", "wvt", "wot"):
                w_sb[n] = wpool.tile([128, NCH, HID], f16, name=n + "_sb", tag=n)
            xt_r = xt.rearrange("(c p) t -> p c t", p=128)

            # first x tiles jump the queue ahead of the weight bulk
            def load_xT(i):
                t = work.tile([128, NCH, TILE], f16, name=f"xT{i}", tag="xT")
                nc.sync.dma_start(t[:], xt_r[:, :, i * TILE : (i + 1) * TILE])
                return t

            xT_next = load_xT(0)
            k = 0
            for n in ("wqt", "wkt", "wvt", "wot"):
                for c in range(NCH):
                    engs[k % 3].dma_start(
                        w_sb[n][:, c, :],
                        wts[n][c * 128 : (c + 1) * 128, :],
                    )
                    k += 1

            mask_sb = wpool.tile([128, 128], f16, tag="mask")
            nc.scalar.dma_start(mask_sb[:], mask_d[:])
            ident_sb = wpool.tile([128, 128], f16, tag="ident")
            nc.scalar.dma_start(ident_sb[:], ident_d[:])

            prev_abd_inst = None
            for it in range(NT):
                t0 = it * TILE
                xT = xT_next
                if it + 1 < NT:
                    xT_next = load_xT(it + 1)

                # ---- projections q,k,v ----
                ps = {
                    n: [pp.tile([128, 512], f32, name=f"ps{n}{h}", tag=f"ps{n}{h}")
                        for h in range(2)]
                    for n in ("q", "k", "v")
                }
                if it == 0:
                    # weights stream in proj-major; don't let k/v matmuls
                    # (waiting on later weights) block q's in the PE queue
                    order = [(n, wn, c) for n, wn in
                             (("q", "wqt"), ("k", "wkt"), ("v", "wvt"))
                             for c in range(NCH)]
                else:
                    order = [(n, wn, c) for c in range(NCH) for n, wn in
                             (("q", "wqt"), ("k", "wkt"), ("v", "wvt"))]
                for n, wn, c in order:
                    for h in range(2):
                        nc.tensor.matmul(
                            ps[n][h][:],
                            xT[:, c, :],
                            w_sb[wn][:, c, h * 512 : (h + 1) * 512],
                            start=(c == 0),
                            stop=(c == NCH - 1),
                        )
                q_sb = work.tile([128, HID], f16, tag="q")
                k_sb = work.tile([128, HID], f16, tag="k")
                # comb packs [A (16) | v (64)] per head-group g so one scatter
                # DMA per token-group moves both to (b,g)-partition layout
                comb = work.tile([128, NH, 16 + HD], f16, tag="comb")
                for h in range(2):
                    nc.scalar.copy(q_sb[:, h * 512 : (h + 1) * 512], ps["q"][h][:])
                    nc.scalar.copy(k_sb[:, h * 512 : (h + 1) * 512], ps["k"][h][:])
                    nc.scalar.copy(
                        comb[:, h * 8 : (h + 1) * 8, 16:],
                        ps["v"][h][:].rearrange("p (g d) -> p g d", g=8),
                    )
                v_sb = comb[:, :, 16:]

                # ---- scores, g-major: prod[t,(g,h,d)] = k[t,(g,d)] * q[t,(h,d)]
                prod = prodp.tile([128, NH, NH, HD], f16, tag="prod")
                q_ap = (
                    q_sb[:]
                    .rearrange("p (h d) -> p h d", h=NH)
                    .unsqueeze(1)
                    .broadcast_to((128, NH, NH, HD))
                )
                k_ap = (
                    k_sb[:]
                    .rearrange("p (g d) -> p g d", g=NH)
                    .unsqueeze(2)
                    .broadcast_to((128, NH, NH, HD))
                )
                for gh in range(2):
                    prod_inst = nc.vector.tensor_tensor(
                        prod[:, gh * 8 : (gh + 1) * 8, :, :],
                        k_ap[:, gh * 8 : (gh + 1) * 8, :, :],
                        q_ap[:, gh * 8 : (gh + 1) * 8, :, :],
                        op=OP.mult,
                    )
                    if gh == 0 and prev_abd_inst is not None:
                        # keep older tiles' attention tail ahead of newer
                        # tiles' big scores op in the DVE queue
                        add_dep_helper(prev_abd_inst.ins, prod_inst.ins,
                                       sync=False, reason="pipeline throttle")

                # pairwise-add tree over d (all ops 2B + step1 => DVE 2x)
                p3 = prod[:].rearrange("p g h d -> p (g h) d")
                scrA = prodp.tile([128, NH * NH, 32], f16, tag="scrA")
                scrB = prodp.tile([128, NH * NH, 16], f16, tag="scrB")
                with nc.allow_low_precision(reason="fp16 score partials"):
                    nc.vector.tensor_tensor(
                        scrA[:], p3[:, :, 0:32], p3[:, :, 32:64], op=OP.add
                    )
                    nc.vector.tensor_tensor(
                        scrB[:], scrA[:, :, 0:16], scrA[:, :, 16:32], op=OP.add
                    )
                    nc.vector.tensor_tensor(
                        scrA[:, :, 0:8], scrB[:, :, 0:8], scrB[:, :, 8:16], op=OP.add
                    )
                    nc.vector.tensor_tensor(
                        scrB[:, :, 0:4], scrA[:, :, 0:4], scrA[:, :, 4:8], op=OP.add
                    )
                    nc.vector.tensor_tensor(
                        scrA[:, :, 0:2], scrB[:, :, 0:2], scrB[:, :, 2:4], op=OP.add
                    )
                    scores = work.tile([128, NH * NH], f16, tag="scores")
                    nc.vector.tensor_tensor(
                        scores[:].unsqueeze(2),
                        scrA[:, :, 0:1],
                        scrA[:, :, 1:2],
                        op=OP.add,
                    )

                # ---- softmax over g (scores laid out (g,h)) ----
                ex = work.tile([128, NH * NH], f16, tag="ex")
                nc.scalar.activation(ex[:], scores[:], AF.Exp, scale=0.125)
                ssum = work.tile([128, NH], f32, tag="ssum")
                ex_hg = ex[:].rearrange("p (g h) -> p h g", g=NH)  # strided view
                nc.vector.tensor_reduce(ssum[:], ex_hg, axis=AX.X, op=OP.add)
                rs = work.tile([128, NH], f32, tag="rs")
                nc.vector.reciprocal(rs[:], ssum[:])
                attw = comb[:, :, 0:16]  # (g, h) slot of comb
                nc.vector.tensor_tensor(
                    attw,
                    ex[:].rearrange("p (g h) -> p g h", g=NH),
                    rs[:].unsqueeze(1).broadcast_to((128, NH, NH)),
                    op=OP.mult,
                )

                # ---- AV on PE ----
                # K-partition index (b,g), b = token-within-contiguous-8-group.
                # One scatter per 8-token group moves [A|v] to (b,g)-partition
                # layout (SWDGE queue); DVE broadcasts A over b' and masks to
                # block-diagonal; one 128x128 @ 128x64 matmul = 8 tokens.
                comb_k = work.tile([128, NG, 16 + HD], f16, tag="comb_k")
                abd_m = prodp.tile([128, NG, 8, NH], f16, tag="abd_m")
                attn_pm = work.tile([128, NG, HD], f16, tag="attn_pm")
                attn16 = work.tile([128, HID], f16, tag="attn16")
                mask_ap = (
                    mask_sb[:]
                    .rearrange("p (b h) -> p b h", b=8)
                    .unsqueeze(1)
                    .broadcast_to((128, NG // 2, 8, NH))
                )
                pa = [pp.tile([128, NG // 2, HD], f32, name=f"pa{i}", tag=f"pav{i}")
                      for i in range(2)]
                for i in range(2):
                    g0 = i * (NG // 2)
                    for j in range(NG // 2):
                        grp = g0 + j
                        eng = (nc.sync, nc.gpsimd, nc.scalar)[grp % 3]
                        eng.dma_start(
                            comb_k[:, grp, :], comb[grp * 8 : (grp + 1) * 8, :, :]
                        )
                    sl = slice(g0, g0 + NG // 2)
                    prev_abd_inst = nc.vector.tensor_tensor(
                        abd_m[:, sl],
                        comb_k[:, sl, 0:16]
                        .unsqueeze(2)
                        .broadcast_to((128, NG // 2, 8, NH)),
                        mask_ap,
                        op=OP.mult,
                    )
                    for j in range(NG // 2):
                        grp = g0 + j
                        nc.tensor.matmul(
                            pa[i][:, j, :],
                            abd_m[:, grp, :, :].rearrange("p b h -> p (b h)"),
                            comb_k[:, grp, 16:],
                            start=True,
                            stop=True,
                        )
                    nc.scalar.copy(attn_pm[:, sl, :], pa[i][:])
                    for j in range(NG // 2):
                        grp = g0 + j
                        eng = (nc.gpsimd, nc.scalar, nc.sync)[grp % 3]
                        eng.dma_start(
                            attn16[grp * 8 : (grp + 1) * 8, :].rearrange(
                                "t (h d) -> t h d", h=NH
                            ),
                            attn_pm[:, grp, :],
                        )

                # ---- output projection (oT via PE transpose) ----
                oT = work.tile([128, NCH, TILE], f16, tag="oT")
                for half in range(2):
                    pt = pp.tile([128, 512], f16, name=f"pt{half}", tag=f"pav{half}")
                    for j in range(4):
                        c = half * 4 + j
                        nc.tensor.transpose(
                            pt[:, j * 128 : (j + 1) * 128],
                            attn16[:, c * 128 : (c + 1) * 128],
                            ident_sb[:],
                        )
                    nc.scalar.copy(
                        oT[:, half * 4 : (half + 1) * 4, :].rearrange(
                            "p c t -> p (c t)"
                        ),
                        pt[:],
                    )
                py = [pp.tile([128, 512], f32, name=f"py{h}", tag=f"pav{h}")
                      for h in range(2)]
                for c in range(NCH):
                    for h in range(2):
                        nc.tensor.matmul(
                            py[h][:],
                            oT[:, c, :],
                            w_sb["wot"][:, c, h * 512 : (h + 1) * 512],
                            start=(c == 0),
                            stop=(c == NCH - 1),
                        )
                y_sb = work.tile([128, HID], f16, tag="ysb")
                for h in range(2):
                    nc.scalar.copy(y_sb[:, h * 512 : (h + 1) * 512], py[h][:])
                nc.sync.dma_start(y[t0 : t0 + TILE, :], y_sb[:])

    nc.compile()
    _cache["nc"] = nc


def _prep_inputs(x, wq, wk, wv, wo):
    x2 = np.asarray(x, dtype=np.float32).reshape(-1, HID)
    w16 = {
        n: np.ascontiguousarray(np.asarray(w, dtype=np.float32).T).astype(np.float16)
        for n, w in (("wqt", wq), ("wkt", wk), ("wvt", wv), ("wot", wo))
    }
    mask = np.zeros((8, 16, 8, 16), dtype=np.float16)
    for b in range(8):
        mask[b, :, b, :] = 1.0
    mask = mask.reshape(128, 128)
    in_maps = []
    for i in range(N_CORES):
        sh = x2[i * TPC : (i + 1) * TPC].astype(np.float16)
        m = {"xt": np.ascontiguousarray(sh.T), "mask": mask,
             "ident": np.eye(128, dtype=np.float16)}
        m.update(w16)
        in_maps.append(m)
    return in_maps


def kernel(x, wq, wk, wv, wo, _trace=False):
    from concourse import bass_utils

    _build()
    in_maps = _prep_inputs(x, wq, wk, wv, wo)
    res = bass_utils.run_bass_kernel_spmd(
        _cache["nc"], in_maps, core_ids=list(range(N_CORES)), trace=_trace
    )
    kernel.last_result = res
    B, S = 4, 4096
    out = np.concatenate([r["y"] for r in res.results], axis=0)
    return out.reshape(B, S, HID).astype(np.float32)


# revision 34
# speedup vs baseline: 1.0140x; 1.0140x over previous
"""Trainium2 Bass kernel for nn_Attention_63934883168998.

Math (per token t): q,k,v = x W{q,k,v}^T reshaped (16 heads, 64); scores over
HEADS: S = q k^T / 8 (16x16), A = softmax(S), out = A v -> (1024); y = out Wo^T.

Sharding: pure data parallel over the 16384 tokens -> 2048 tokens/core.
All on-chip data fp16 (PE fp16 matmul = full rate, ~5e-4 GEMM rel-err).

Per 128-token tile:
  - x^T arrives pre-transposed from the host; projections on PE with
    stationary = x^T chunk, moving = host-pretransposed W^T -> q,k,v in
    natural (token-partition) layout.
  - scores S[t,(g,h)] = sum_d q*k on the Vector engine: one broadcast-AP
    tensor_tensor (2x mode) + a log2(64)-pass pairwise-add tree (2x mode);
    softmax on ScalarE/Vector.  g-major layout so the A-scatter below has
    contiguous 32B runs.
  - AV combine on the TENSOR engine: stationary = 128x128 block-diagonal A
    for 8 tokens (K=(tau,g)), moving = v gathered to (tau,g)-partition
    layout; off-diagonal zeros kill cross-token terms.  Operands built by
    partition-scatter DMAs (plain, 32B/128B runs).
  - attention out scattered back to token-partition layout, DMA-transposed
    to feature-partition chunks, final projection on PE.
"""

import numpy as np
import ml_dtypes

BF16 = ml_dtypes.bfloat16
N_CORES = 8
HID = 1024
NH, HD = 16, 64
TILE = 128
TPC = 16384 // N_CORES      # tokens per core
NT = TPC // TILE            # token tiles per core
NCH = HID // 128            # 8 hidden chunks
NG = TILE // 8              # 16 groups of 8 tokens

_cache = {}


def _build():
    if "nc" in _cache:
        return
    import concourse.bacc as bacc
    import concourse.mybir as mybir
    from concourse import tile
    from concourse.tile import add_dep_helper

    f16 = mybir.dt.float16
    bf16 = mybir.dt.bfloat16
    f32 = mybir.dt.float32
    AX = mybir.AxisListType
    OP = mybir.AluOpType
    AF = mybir.ActivationFunctionType

    nc = bacc.Bacc("TRN2", target_bir_lowering=False, debug=False)
    xt = nc.dram_tensor("xt", (HID, TPC), f16, kind="ExternalInput").ap()
    wts = {
        n: nc.dram_tensor(n, (HID, HID), f16, kind="ExternalInput").ap()
        for n in ("wqt", "wkt", "wvt", "wot")
    }
    # block-diag 0/1 mask: mask[(b,g),(b',h)] = (b == b')
    mask_d = nc.dram_tensor("mask", (128, 128), f16, kind="ExternalInput").ap()
    ident_d = nc.dram_tensor("ident", (128, 128), f16, kind="ExternalInput").ap()
    y = nc.dram_tensor("y", (TPC, HID), f16, kind="ExternalOutput").ap()

    with tile.TileContext(nc) as tc:
        with (
            tc.tile_pool(name="wpool", bufs=1) as wpool,
            tc.tile_pool(name="work", bufs=3) as work,
            tc.tile_pool(name="prod", bufs=1) as prodp,
            tc.tile_pool(name="psum", bufs=1, space="PSUM") as pp,
        ):
            # Resident weights: chunk c = W^T[c*128:(c+1)*128, :]
            w_sb = {}
            engs = [nc.sync, nc.scalar, nc.gpsimd]
            for n in ("wqt", "wkt", "wvt", "wot"):
                w_sb[n] = wpool.tile([128, NCH, HID], f16, name=n + "_sb", tag=n)
            xt_r = xt.rearrange("(c p) t -> p c t", p=128)

            # first x tiles jump the queue ahead of the weight bulk
            def load_xT(i):
                t = work.tile([128, NCH, TILE], f16, name=f"xT{i}", tag="xT")
                nc.sync.dma_start(t[:], xt_r[:, :, i * TILE : (i + 1) * TILE])
                return t

            xT_next = load_xT(0)
            k = 0
            for n in ("wqt", "wkt", "wvt", "wot"):
                for c in range(NCH):
                    engs[k % 3].dma_start(
                        w_sb[n][:, c, :],
                        wts[n][c * 128 : (c + 1) * 128, :],
                    )
                    k += 1

            mask_sb = wpool.tile([128, 128], f16, tag="mask")
            nc.scalar.dma_start(mask_sb[:], mask_d[:])
            ident_sb = wpool.tile([128, 128], f16, tag="ident")
            nc.scalar.dma_start(ident_sb[:], ident_d[:])

            prev_abd_inst = None
            for it in range(NT):
                t0 = it * TILE
                xT = xT_next
                if it + 1 < NT:
                    xT_next = load_xT(it + 1)

                # ---- projections q,k,v ----
                ps = {
                    n: [pp.tile([128, 512], f32, name=f"ps{n}{h}", tag=f"ps{n}{h}")
                        for h in range(2)]
                    for n in ("q", "k", "v")
                }
                if it == 0:
                    # weights stream in proj-major; don't let k/v matmuls
                    # (waiting on later weights) block q's in the PE queue
                    order = [(n, wn, c) for n, wn in
                             (("q", "wqt"), ("k", "wkt"), ("v", "wvt"))
                             for c in range(NCH)]
                else:
                    order = [(n, wn, c) for c in range(NCH) for n, wn in
                             (("q", "wqt"), ("k", "wkt"), ("v", "wvt"))]
                for n, wn, c in order:
                    for h in range(2):
                        nc.tensor.matmul(
                            ps[n][h][:],
                            xT[:, c, :],
                            w_sb[wn][:, c, h * 512 : (h + 1) * 512],
                            start=(c == 0),
                            stop=(c == NCH - 1),
                        )
                q_sb = work.tile([128, HID], f16, tag="q")
                k_sb = work.tile([128, HID], f16, tag="k")
                # comb packs [A (16) | v (64)] per head-group g so one scatter
                # DMA per token-group moves both to (b,g)-partition layout
                comb = work.tile([128, NH, 16 + HD], f16, tag="comb")
                for h in range(2):
                    nc.scalar.copy(q_sb[:, h * 512 : (h + 1) * 512], ps["q"][h][:])
                    nc.scalar.copy(k_sb[:, h * 512 : (h + 1) * 512], ps["k"][h][:])
                    nc.scalar.copy(
                        comb[:, h * 8 : (h + 1) * 8, 16:],
                        ps["v"][h][:].rearrange("p (g d) -> p g d", g=8),
                    )
                v_sb = comb[:, :, 16:]

                # ---- scores, g-major: prod[t,(g,h,d)] = k[t,(g,d)] * q[t,(h,d)]
                prod = prodp.tile([128, NH, NH, HD], f16, tag="prod")
                q_ap = (
                    q_sb[:]
                    .rearrange("p (h d) -> p h d", h=NH)
                    .unsqueeze(1)
                    .broadcast_to((128, NH, NH, HD))
                )
                k_ap = (
                    k_sb[:]
                    .rearrange("p (g d) -> p g d", g=NH)
                    .unsqueeze(2)
                    .broadcast_to((128, NH, NH, HD))
                )
                prod_inst = nc.vector.tensor_tensor(prod[:], k_ap, q_ap, op=OP.mult)
                if prev_abd_inst is not None:
                    # keep older tiles' attention tail ahead of newer tiles'
                    # big scores op in the DVE queue
                    add_dep_helper(prev_abd_inst.ins, prod_inst.ins, sync=False,
                                   reason="pipeline throttle")

                # pairwise-add tree over d (all ops 2B + step1 => DVE 2x)
                p3 = prod[:].rearrange("p g h d -> p (g h) d")
                scrA = prodp.tile([128, NH * NH, 32], f16, tag="scrA")
                scrB = prodp.tile([128, NH * NH, 16], f16, tag="scrB")
                with nc.allow_low_precision(reason="fp16 score partials"):
                    nc.vector.tensor_tensor(
                        scrA[:], p3[:, :, 0:32], p3[:, :, 32:64], op=OP.add
                    )
                    nc.vector.tensor_tensor(
                        scrB[:], scrA[:, :, 0:16], scrA[:, :, 16:32], op=OP.add
                    )
                    nc.vector.tensor_tensor(
                        scrA[:, :, 0:8], scrB[:, :, 0:8], scrB[:, :, 8:16], op=OP.add
                    )
                    nc.vector.tensor_tensor(
                        scrB[:, :, 0:4], scrA[:, :, 0:4], scrA[:, :, 4:8], op=OP.add
                    )
                    nc.vector.tensor_tensor(
                        scrA[:, :, 0:2], scrB[:, :, 0:2], scrB[:, :, 2:4], op=OP.add
                    )
                    scores = work.tile([128, NH * NH], f16, tag="scores")
                    nc.vector.tensor_tensor(
                        scores[:].unsqueeze(2),
                        scrA[:, :, 0:1],
                        scrA[:, :, 1:2],
                        op=OP.add,
                    )

                # ---- softmax over g (scores laid out (g,h)) ----
                ex = work.tile([128, NH * NH], f16, tag="ex")
                nc.scalar.activation(ex[:], scores[:], AF.Exp, scale=0.125)
                ssum = work.tile([128, NH], f32, tag="ssum")
                ex_hg = ex[:].rearrange("p (g h) -> p h g", g=NH)  # strided view
                nc.vector.tensor_reduce(ssum[:], ex_hg, axis=AX.X, op=OP.add)
                rs = work.tile([128, NH], f32, tag="rs")
                nc.vector.reciprocal(rs[:], ssum[:])
                attw = comb[:, :, 0:16]  # (g, h) slot of comb
                nc.vector.tensor_tensor(
                    attw,
                    ex[:].rearrange("p (g h) -> p g h", g=NH),
                    rs[:].unsqueeze(1).broadcast_to((128, NH, NH)),
                    op=OP.mult,
                )

                # ---- AV on PE ----
                # K-partition index (b,g), b = token-within-contiguous-8-group.
                # One scatter per 8-token group moves [A|v] to (b,g)-partition
                # layout (SWDGE queue); DVE broadcasts A over b' and masks to
                # block-diagonal; one 128x128 @ 128x64 matmul = 8 tokens.
                comb_k = work.tile([128, NG, 16 + HD], f16, tag="comb_k")
                for grp in range(NG):
                    eng = (nc.sync, nc.gpsimd, nc.scalar)[grp % 3]
                    eng.dma_start(
                        comb_k[:, grp, :], comb[grp * 8 : (grp + 1) * 8, :, :]
                    )
                abd_m = prodp.tile([128, NG, 8, NH], f16, tag="abd_m")
                prev_abd_inst = nc.vector.tensor_tensor(
                    abd_m[:],
                    comb_k[:, :, 0:16].unsqueeze(2).broadcast_to((128, NG, 8, NH)),
                    mask_sb[:]
                    .rearrange("p (b h) -> p b h", b=8)
                    .unsqueeze(1)
                    .broadcast_to((128, NG, 8, NH)),
                    op=OP.mult,
                )
                pa = [pp.tile([128, NG // 2, HD], f32, name=f"pa{i}", tag=f"pav{i}")
                      for i in range(2)]
                for grp in range(NG):
                    nc.tensor.matmul(
                        pa[grp // 8][:, grp % 8, :],
                        abd_m[:, grp, :, :].rearrange("p b h -> p (b h)"),
                        comb_k[:, grp, 16:],
                        start=True,
                        stop=True,
                    )
                # attn in ((b,h), grp, d) partition-interleaved layout
                attn_pm = work.tile([128, NG, HD], f16, tag="attn_pm")
                for i in range(2):
                    nc.scalar.copy(attn_pm[:, i * 8 : (i + 1) * 8, :], pa[i][:])

                # scatter back to token-partition natural layout
                attn16 = work.tile([128, HID], f16, tag="attn16")
                for grp in range(NG):
                    eng = (nc.gpsimd, nc.scalar, nc.sync)[grp % 3]
                    eng.dma_start(
                        attn16[grp * 8 : (grp + 1) * 8, :].rearrange(
                            "t (h d) -> t h d", h=NH
                        ),
                        attn_pm[:, grp, :],
                    )

                # ---- output projection (oT via PE transpose) ----
                oT = work.tile([128, NCH, TILE], f16, tag="oT")
                for half in range(2):
                    pt = pp.tile([128, 512], f16, name=f"pt{half}", tag=f"pav{half}")
                    for j in range(4):
                        c = half * 4 + j
                        nc.tensor.transpose(
                            pt[:, j * 128 : (j + 1) * 128],
                            attn16[:, c * 128 : (c + 1) * 128],
                            ident_sb[:],
                        )
                    nc.scalar.copy(
                        oT[:, half * 4 : (half + 1) * 4, :].rearrange(
                            "p c t -> p (c t)"
                        ),
                        pt[:],
                    )
                py = [pp.tile([128, 512], f32, name=f"py{h}", tag=f"pav{h}")
                      for h in range(2)]
                for c in range(NCH):
                    for h in range(2):
                        nc.tensor.matmul(
                            py[h][:],
                            oT[:, c, :],
                            w_sb["wot"][:, c, h * 512 : (h + 1) * 512],
                            start=(c == 0),
                            stop=(c == NCH - 1),
                        )
                y_sb = work.tile([128, HID], f16, tag="ysb")
                for h in range(2):
                    nc.scalar.copy(y_sb[:, h * 512 : (h + 1) * 512], py[h][:])
                nc.sync.dma_start(y[t0 : t0 + TILE, :], y_sb[:])

    nc.compile()
    _cache["nc"] = nc


def _prep_inputs(x, wq, wk, wv, wo):
    x2 = np.asarray(x, dtype=np.float32).reshape(-1, HID)
    w16 = {
        n: np.ascontiguousarray(np.asarray(w, dtype=np.float32).T).astype(np.float16)
        for n, w in (("wqt", wq), ("wkt", wk), ("wvt", wv), ("wot", wo))
    }
    mask = np.zeros((8, 16, 8, 16), dtype=np.float16)
    for b in range(8):
        mask[b, :, b, :] = 1.0
    mask = mask.reshape(128, 128)
    in_maps = []
    for i in range(N_CORES):
        sh = x2[i * TPC : (i + 1) * TPC].astype(np.float16)
        m = {"xt": np.ascontiguousarray(sh.T), "mask": mask,
             "ident": np.eye(128, dtype=np.float16)}
        m.update(w16)
        in_maps.append(m)
    return in_maps


def kernel(x, wq, wk, wv, wo, _trace=False):
    from concourse import bass_utils

    _build()
    in_maps = _prep_inputs(x, wq, wk, wv, wo)
    res = bass_utils.run_bass_kernel_spmd(
        _cache["nc"], in_maps, core_ids=list(range(N_CORES)), trace=_trace
    )
    kernel.last_result = res
    B, S = 4, 4096
    out = np.concatenate([r["y"] for r in res.results], axis=0)
    return out.reshape(B, S, HID).astype(np.float32)
